# revision 10
# baseline (speedup 1.0000x reference)
"""FFM (field-aware factorization machine) forward pass on 8 Trainium2 cores.

Math (per sample b):
    linear[b] = X[b,:] @ w1 + b0
    C[i,j]    = sum_k v[i, field[j], k] * v[j, field[i], k]   (pair coefficients)
    inter[b]  = sum_{i<j} C[i,j] X[b,i] X[b,j]
    out[b]    = sigmoid(linear[b] + inter[b])

Strategy (v2 -- fp8 DoubleRow + split epilogue + issue-cheap DMA):
  * Precompute Cm = strict-upper(C) on host, fold w1^T into the (structurally
    zero) row 511, scale by 2^SC into fp8e4m3 range; X also goes to fp8 for
    the matmul (plus a bf16 natural-layout copy for the epilogue).  The
    sigmoid's free scale argument multiplies by 2^-SC at the end.
  * Y = X@Cm via fp8 DoubleRow matmuls: contraction 512 = 2 chunks of 256
    (2 packed k-tiles each).  Chunk A = k-tiles {0,3} (full width, carries
    the w1 row), chunk B = k-tiles {1,2} (strict-upper trim: cols 128..511).
    2 matmuls/tile instead of 4, at ~2x fp8 element rate.
  * Split epilogue: rowsum(Y*X) with the PSUM drain shared between engines
    by TILE PARITY (ScalarE+VectorE may not touch the same psum bank
    concurrently -- hw collision abort).  Even tiles: VectorE drains psum
    directly with one fp32 STT.  Odd tiles: ScalarE (closer to PSUM) copies
    the whole bank to bf16 SBUF; VectorE then multiplies it against X in
    2x-mode bf16.  One shared accumulator array, no merge step (reading a
    DVE accum_out from the NEXT DVE op races the accumulator flush; all acc
    reads here are cross-engine behind semaphores).  ScalarE also does the
    batched sigmoids.
  * All DRAM layouts are per-partition contiguous so each dma_start emits
    128 large descriptors (the baseline's strided layouts cost ~1.1us of
    sequencer DIRECT2D per dma_start).  Issue lanes: sync = C + X^T groups
    + outputs, scalar = bias + first natural-X groups (before its copy loop
    starts), gpsimd (SWDGE) = remaining natural-X groups.
  * A few dummy DoubleRow matmuls at stream start warm the PE HAM clock
    gate while the first DMA groups land.

Raw bass (no TileContext: this container's walrus rejects Tile's multi-wait
encodings and the TENSOR_TENSOR_REDUCE direct-ISA opcode).
"""

import contextlib

import numpy as np
import ml_dtypes

P = 128          # partitions / tile rows
F = 512          # features
NCORES = 8
B = 32768
BSH = B // NCORES   # 4096 rows per core
NT = BSH // P       # 32 batch tiles per core
NPAIR = 3           # psum bank-pair rotation depth (2 banks each)
NWARM = 6           # dummy warm-up matmuls bridging the first DMA arrivals
KM = ((0, 3), (1, 2))   # k-tile pairing for the two DoubleRow chunks
CB_J0 = 128             # chunk B column base (strict-upper trim)

BF16 = ml_dtypes.bfloat16
FP8 = ml_dtypes.float8_e4m3


def _groups(singles, pairs_until, quad):
    gs = [(t, 1) for t in range(singles)]
    t = singles
    while t < pairs_until:
        gs.append((t, 2))
        t += 2
    while t < NT:
        n = min(quad, NT - t)
        gs.append((t, n))
        t += n
    return gs


XT_G = _groups(2, 6, 8)     # [(0,1),(1,1),(2,2),(4,2),(6,8),(14,8),(22,8),(30,2)]
XN_G = _groups(2, 6, 8)
N_XN_SCALAR = 4             # xn groups issued on the scalar queue (the rest: sync)
# NOTE: gpsimd (SWDGE) dma_start hangs multi-core runs in this container --
# all DMA goes through the two HWDGE lanes (sync, scalar).


def _build_bass(sc_pow):
    import concourse.bass as bass
    from concourse import mybir

    nc = bass.Bass()

    xn_d = nc.declare_dram_parameter("xn", [P, NT, F], mybir.dt.bfloat16, isOutput=False)[:]
    xt_d = nc.declare_dram_parameter("xt", [P, NT, 2, 2, P], mybir.dt.float8e4, isOutput=False)[:]
    ca_d = nc.declare_dram_parameter("ca", [P, 2, F], mybir.dt.float8e4, isOutput=False)[:]
    cb_d = nc.declare_dram_parameter("cb", [P, 2, F - CB_J0], mybir.dt.float8e4, isOutput=False)[:]
    bias = nc.declare_dram_parameter("bias", [1], mybir.dt.float32, isOutput=False)[:]
    y = nc.declare_dram_parameter("y", [P, NT], mybir.dt.float32, isOutput=True)[:]

    xt_of = {}
    for gi, (t0, n) in enumerate(XT_G):
        for t in range(t0, t0 + n):
            xt_of[t] = gi
    xn_of = {}
    for gi, (t0, n) in enumerate(XN_G):
        for t in range(t0, t0 + n):
            xn_of[t] = gi

    DR = mybir.MatmulPerfMode.DoubleRow

    with contextlib.ExitStack() as st:
        ec = st.enter_context
        ca_sb = ec(nc.sbuf_tensor([P, 2, F], mybir.dt.float8e4))
        cb_sb = ec(nc.sbuf_tensor([P, 2, F - CB_J0], mybir.dt.float8e4))
        xbuf = ec(nc.sbuf_tensor([P, NT, F], mybir.dt.bfloat16))
        xtbuf = ec(nc.sbuf_tensor([P, NT, 2, 2, P], mybir.dt.float8e4))
        ycopy = ec(nc.sbuf_tensor([P, 2, F], mybir.dt.bfloat16))
        dump = ec(nc.sbuf_tensor([P, F], mybir.dt.bfloat16))
        acc1 = ec(nc.sbuf_tensor([P, NT], mybir.dt.float32))
        out_sb = ec(nc.sbuf_tensor([P, NT], mybir.dt.float32))
        b_sb = ec(nc.sbuf_tensor([P, 1], mybir.dt.float32))
        pp = [ec(nc.psum_tensor(f"pp{i}", [P, 2, F], mybir.dt.float32)) for i in range(NPAIR)]
        ps_warm = ec(nc.psum_tensor("ps_warm", [P, F], mybir.dt.float32))

        s_ca = ec(nc.semaphore(name="s_ca"))
        s_cb = ec(nc.semaphore(name="s_cb"))
        s_b = ec(nc.semaphore(name="s_b"))
        s_xt = [ec(nc.semaphore(name=f"s_xt{i}")) for i in range(len(XT_G))]
        s_xn = [ec(nc.semaphore(name=f"s_xn{i}")) for i in range(len(XN_G))]
        s_mm = ec(nc.semaphore(name="s_mm"))    # +1 per tile (tensor)
        s_cp = ec(nc.semaphore(name="s_cp"))    # +1 per odd-tile scalar copy
        s_d2 = ec(nc.semaphore(name="s_d2"))    # +1 per pair fully drained by DVE
        s_act = ec(nc.semaphore(name="s_act"))
        s_out = ec(nc.semaphore(name="s_out"))

        block = ec(nc.Block())

        @block.sync
        def _(sync):
            sync.dma_start(out=ca_sb[:], in_=ca_d).then_inc(s_ca, 16)
            sync.dma_start(out=cb_sb[:], in_=cb_d).then_inc(s_cb, 16)
            # interleave xt / late-xn groups by first-use time
            xn_left = list(range(N_XN_SCALAR, len(XN_G)))
            for gi, (t0, n) in enumerate(XT_G):
                sync.dma_start(
                    out=xtbuf[:, t0 : t0 + n], in_=xt_d[:, t0 : t0 + n]
                ).then_inc(s_xt[gi], 16)
                while xn_left and XN_G[xn_left[0]][0] <= t0:
                    gj = xn_left.pop(0)
                    u0, un = XN_G[gj]
                    sync.dma_start(
                        out=xbuf[:, u0 : u0 + un], in_=xn_d[:, u0 : u0 + un]
                    ).then_inc(s_xn[gj], 16)
            for gj in xn_left:
                u0, un = XN_G[gj]
                sync.dma_start(
                    out=xbuf[:, u0 : u0 + un], in_=xn_d[:, u0 : u0 + un]
                ).then_inc(s_xn[gj], 16)
            # outputs: every 8 tiles = 2 sigmoid groups
            for yo in range(4):
                sync.wait_ge(s_act, 2 * (yo + 1))
                sync.dma_start(
                    out=y[:, 8 * yo : 8 * yo + 8], in_=out_sb[:, 8 * yo : 8 * yo + 8]
                ).then_inc(s_out, 16)
            sync.wait_ge(s_out, 64)

        @block.scalar
        def _(scalar):
            scalar.dma_start(out=b_sb[:], in_=bias.to_broadcast([P, 1])).then_inc(s_b, 16)
            for gi in range(N_XN_SCALAR):
                t0, n = XN_G[gi]
                scalar.dma_start(
                    out=xbuf[:, t0 : t0 + n], in_=xn_d[:, t0 : t0 + n]
                ).then_inc(s_xn[gi], 16)
            scalar.wait_ge(s_b, 16)
            for q in range(NT // 2):        # odd tile of each psum pair
                t = 2 * q + 1
                scalar.wait_ge(s_mm, t + 1)
                if q >= 2:
                    # ycopy 2-slot rotation: slot q%2 last held pair q-2
                    scalar.wait_ge(s_d2, q - 1)
                nc.scalar.activation(
                    out=ycopy[:, q % 2, :],
                    in_=pp[q % NPAIR][:, 1, :],
                    func=mybir.ActivationFunctionType.Copy,
                ).then_inc(s_cp, 1)
                if q % 2 == 1:
                    m = q // 2
                    scalar.wait_ge(s_d2, 2 * m + 2)
                    nc.scalar.activation(
                        out=out_sb[:, 4 * m : 4 * m + 4],
                        in_=acc1[:, 4 * m : 4 * m + 4],
                        func=mybir.ActivationFunctionType.Sigmoid,
                        bias=b_sb[:],
                        scale=float(2.0 ** (-sc_pow)),
                    ).then_inc(s_act, 1)

        @block.tensor
        def _(tensor):
            for _w in range(NWARM):
                nc.tensor.matmul(
                    ps_warm[:],
                    xtbuf[:, 0, 0, :, :],
                    ca_sb[:, :, :],
                    start=True,
                    stop=True,
                    perf_mode=DR,
                    skip_group_check=True,
                )
            tensor.wait_ge(s_ca, 16)
            tensor.wait_ge(s_cb, 16)
            for t in range(NT):
                gi = xt_of[t]
                if t == XT_G[gi][0]:
                    tensor.wait_ge(s_xt[gi], 16)
                q = t // 2
                if t % 2 == 0 and q >= NPAIR:
                    # reuse of psum pair slot: previous occupant pair q-NPAIR
                    tensor.wait_ge(s_cp, q - NPAIR + 1)
                    tensor.wait_ge(s_d2, q - NPAIR + 1)
                pst = pp[q % NPAIR]
                nc.tensor.matmul(
                    pst[:, t % 2, :],
                    xtbuf[:, t, 0, :, :],
                    ca_sb[:, :, :],
                    start=True,
                    stop=False,
                    perf_mode=DR,
                    skip_group_check=True,
                )
                mm = nc.tensor.matmul(
                    pst[:, t % 2, CB_J0:],
                    xtbuf[:, t, 1, :, :],
                    cb_sb[:, :, :],
                    start=False,
                    stop=True,
                    perf_mode=DR,
                    skip_group_check=True,
                )
                mm.then_inc(s_mm, 1)

        @block.vector
        def _(vector):
            for t in range(NT):
                gi = xn_of[t]
                if t == XN_G[gi][0]:
                    vector.wait_ge(s_xn[gi], 16)
                q = t // 2
                # ScalarE+VectorE never touch the same psum bank: DVE reads
                # only bank 0 of each pair, ScalarE only bank 1.
                if t % 2 == 0:
                    vector.wait_ge(s_mm, t + 1)
                    nc.vector.scalar_tensor_tensor(
                        out=dump[:],
                        in0=pp[q % NPAIR][:, 0, :],
                        scalar=0.0,
                        in1=xbuf[:, t, :],
                        op0=mybir.AluOpType.add,
                        op1=mybir.AluOpType.mult,
                        accum_out=acc1[:, t : t + 1],
                    )
                else:
                    vector.wait_ge(s_cp, q + 1)
                    nc.vector.scalar_tensor_tensor(
                        out=dump[:],
                        in0=ycopy[:, q % 2, :],
                        scalar=0.0,
                        in1=xbuf[:, t, :],
                        op0=mybir.AluOpType.add,
                        op1=mybir.AluOpType.mult,
                        accum_out=acc1[:, t : t + 1],
                    ).then_inc(s_d2, 1)

    return nc


def _host_prep(X, w1, b, v, feature2field):
    """Returns (sc_pow, per-core input maps)."""
    X = np.asarray(X, dtype=np.float32)
    w1 = np.asarray(w1, dtype=np.float32)
    b = np.asarray(b, dtype=np.float32)
    v = np.asarray(v, dtype=np.float32)
    f2f = np.asarray(feature2field, dtype=np.int32)

    # Pair-coefficient matrix: C[i,j] = sum_k v[i, f2f[j], k] * v[j, f2f[i], k]
    A = v[:, f2f, :]                      # [n, n, k]
    C = (A * A.transpose(1, 0, 2)).sum(axis=2)
    Cm = np.triu(C, 1)
    # Fold the linear term: row F-1 of strict-upper Cm is all zeros.
    Cm[F - 1, :] = w1[:, 0]

    # fp8 scaling: put max|Cm| around 160 (fp8e4m3 max = 240)
    maxabs = float(np.abs(Cm).max())
    sc_pow = int(np.floor(np.log2(160.0 / max(maxabs, 1e-30))))
    C8 = (Cm * (2.0 ** sc_pow)).astype(FP8)

    # chunk layouts: c?[p, kt, j] = C8[KM[c][kt]*P + p, j0 + j]
    C8r = C8.reshape(4, P, F)
    ca = np.ascontiguousarray(
        np.stack([C8r[KM[0][0]], C8r[KM[0][1]]], axis=1)
    )                                     # [P, 2, F]
    cb = np.ascontiguousarray(
        np.stack([C8r[KM[1][0], :, CB_J0:], C8r[KM[1][1], :, CB_J0:]], axis=1)
    )                                     # [P, 2, F-CB_J0]

    X8 = X.astype(FP8)
    Xb = X.astype(BF16)
    in_maps = []
    for c in range(NCORES):
        X8c = X8[c * BSH : (c + 1) * BSH]             # [4096, 512]
        Xbc = Xb[c * BSH : (c + 1) * BSH]
        # xt[p, t, ch, kt, b] = X8c[t*P + b, KM[ch][kt]*P + p]
        x4 = X8c.reshape(NT, P, 4, P)                 # [t, b, ktile, p]
        xt = np.ascontiguousarray(
            np.stack(
                [
                    np.stack([x4[:, :, KM[0][0]], x4[:, :, KM[0][1]]], axis=0),
                    np.stack([x4[:, :, KM[1][0]], x4[:, :, KM[1][1]]], axis=0),
                ],
                axis=0,
            ).transpose(4, 2, 0, 1, 3)                # [p, t, ch, kt, b]
        )
        # w1-fold: stationary row for feature 511 (= chunk 0, kt 1, p 127) := 1.0
        xt[P - 1, :, 0, 1, :] = FP8(1.0)
        # xn[p, t, f] = Xbc[t*P + p, f]
        xn = np.ascontiguousarray(Xbc.reshape(NT, P, F).transpose(1, 0, 2))
        in_maps.append({"xn": xn, "xt": xt, "ca": ca, "cb": cb, "bias": b})
    return sc_pow, in_maps


def _run(prep, trace=False):
    from concourse.bass_utils import run_bass_kernel_spmd

    sc_pow, in_maps = prep
    nc = _build_bass(sc_pow)
    res = run_bass_kernel_spmd(nc, in_maps, core_ids=list(range(NCORES)), trace=trace)
    out = np.concatenate([r["y"].reshape(P, NT).T.reshape(-1) for r in res.results])
    return out, res


def kernel(X, w1, b, v, feature2field):
    prep = _host_prep(X, w1, b, v, feature2field)
    out, _ = _run(prep, trace=False)
    return out.astype(np.float32)


if __name__ == "__main__":
    pass
